# revision 2
# baseline (speedup 1.0000x reference)
"""Trainium2 Bass kernel for nn_CascadeTransformerMM (4-layer ternary-GLU cascade).

v8: deep software pipeline, PE-gap-free steady state:
  - q-pass (RMS-scale + act-quant + transpose) runs 2 tiles ahead; row stats
    for layer l+1 fold into layer l's tile tails, computed in two halves so
    the next layer's first q-passes never wait on tile 7.
  - down-projection lags two tiles behind up-projection on the PE queue; the
    gq magic-round (2x [128,4096] scalar ops) is chunked into [128,512]
    pieces interleaved between the NEXT tile's silu ops so the scalar queue
    never head-of-line-blocks a silu (which gates PE via PSUM rotation).
  - weights stream fp32 (fp16 flips ternary signs near threshold; the
    cascade amplifies that to ~3e-2); ternarize = fp32 -> int16 -> fp8 on
    DVE in [128,1024] units, 2 units interleaved per fg-slot of the
    previous layer's tiles 2..7; staging writes ride gpsimd with lag-1.
  - per-matrix |W|max scales are computed host-side (12 scalars).

Math (per layer, per token row):
  h   = rms_scale * x * rsqrt(mean(x^2) + 1e-6)
  s   = clip(127/(max|h| + 1e-5), 1e-3, 1e3);  q = round(s*h)
  Wt  = clip(round(W * 127/(max|W| + 1e-5)), -1, 1)      (ternary {-1,0,1})
  u   = (q @ Wg_t)/s ; v = (q @ Wu_t)/s ; g = silu(u)*v
  s2  = clip(127/(max|g| + 1e-5), 1e-3, 1e3); gq = round(s2*g)
  x  += (gq @ Wd_t)/s2

Distribution: data-parallel over batch (8 batches -> 8 cores), weights
replicated per core.
"""

import os
import sys

for _p in ("/opt/trn_rl_repo", "/root/.axon_site/_ro/trn_rl_repo"):
    if os.path.isdir(_p) and _p not in sys.path:
        sys.path.insert(0, _p)

import numpy as np
from contextlib import ExitStack

import concourse.bass as bass
import concourse.mybir as mybir
import concourse.tile as tile
from concourse.bass_utils import run_bass_kernel_spmd

dt = mybir.dt
AF = mybir.ActivationFunctionType
ALU = mybir.AluOpType

MAGIC = float(1.5 * 2**23)

D = 1024
F = 4096
L = 4
NCORES = 8
TOK = 1024

NDK = D // 128    # 8 contraction tiles for up-proj
NFT = F // 128    # 32 contraction tiles for down-proj
NFC = F // 512    # 8 free-dim chunks for up-proj
NCH = F // 1024   # 4 column-chunks in the repacked wg/wu layout
NTT = TOK // 128  # 8 token tiles
PRE = 2           # q-pass lookahead depth (tiles)


def _split_excess_waits(nc, max_waits: int = 1) -> int:
    """walrus in this container rejects >1 sync-wait per instruction; split
    extras into standalone event-semaphore waits on the same engine."""
    n = 0
    for func in nc.m.functions:
        for block in func.blocks:
            changed = False
            out = []
            for inst in block.instructions:
                si = getattr(inst, "sync_info", None)
                if si is not None and si.on_wait and len(si.on_wait) > max_waits:
                    waits = list(si.on_wait)
                    for j, w in enumerate(waits[max_waits:]):
                        out.append(
                            mybir.InstEventSemaphore(
                                name=f"{inst.name}-xw{j}",
                                engine=inst.engine,
                                ins=[],
                                outs=[],
                                sync_info=mybir.SyncInfo(on_wait=[w], on_update=[]),
                            )
                        )
                        n += 1
                    inst.sync_info = mybir.SyncInfo(
                        on_wait=waits[:max_waits], on_update=list(si.on_update)
                    )
                    changed = True
                out.append(inst)
            if changed:
                block.instructions = out
    return n


def build(n_cores: int = NCORES, n_tok_tiles: int = NTT, n_layers: int = L) -> bass.Bass:
    nc = bass.Bass(num_devices=n_cores)
    tok = n_tok_tiles * 128
    ntt = n_tok_tiles

    x_ext = nc.declare_dram_parameter("x", [tok, D], dt.float32, isOutput=False)
    rs_ext = nc.declare_dram_parameter("rs", [n_layers, D], dt.float32, isOutput=False)
    # wg/wu repacked host-side to [L, NCH, NDK, 128, 1024]: every (ch, dk)
    # weight tile is one contiguous 512 KB stream unit.
    wg_ext = nc.declare_dram_parameter("wg", [n_layers, NCH, NDK, 128, 1024], dt.float32, isOutput=False)
    wu_ext = nc.declare_dram_parameter("wu", [n_layers, NCH, NDK, 128, 1024], dt.float32, isOutput=False)
    wd_ext = nc.declare_dram_parameter("wd", [n_layers, F, D], dt.float32, isOutput=False)
    # host-computed per-matrix quant scales 127/(max|W|+1e-5), order 3l+{g,u,d}
    wscin_ext = nc.declare_dram_parameter("wscin", [1, 16], dt.float32, isOutput=False)
    out_ext = nc.declare_dram_parameter("out", [tok, D], dt.float32, isOutput=True)

    with tile.TileContext(nc) as tc, ExitStack() as ctx:
        P = ctx.enter_context
        const = P(tc.tile_pool(name="const", bufs=1))
        wpool = P(tc.tile_pool(name="wts", bufs=3))
        xpool = P(tc.tile_pool(name="x1", bufs=4))
        t1pool = P(tc.tile_pool(name="t1", bufs=2))
        qpool = P(tc.tile_pool(name="q", bufs=1))
        qtpool = P(tc.tile_pool(name="qt", bufs=3))
        gpool = P(tc.tile_pool(name="g", bufs=2))
        gqpool = P(tc.tile_pool(name="gq", bufs=1))
        gqtpool = P(tc.tile_pool(name="gqt", bufs=1))
        silupool = P(tc.tile_pool(name="silu", bufs=1))
        xdrpool = P(tc.tile_pool(name="xdr", bufs=1))
        wst = P(tc.tile_pool(name="wst", bufs=3))
        wi16 = P(tc.tile_pool(name="wi16", bufs=1))
        w8s = P(tc.tile_pool(name="w8s", bufs=2))
        sbcpool = P(tc.tile_pool(name="sbc", bufs=2))
        rsrow = P(tc.tile_pool(name="rsrow", bufs=1))
        batch = P(tc.tile_pool(name="batch", bufs=2))
        sc = P(tc.tile_pool(name="sc", bufs=4))
        dram = P(tc.tile_pool(name="dram", bufs=2, space="DRAM"))
        psA = P(tc.tile_pool(name="psA", bufs=2, space="PSUM"))
        psB = P(tc.tile_pool(name="psB", bufs=3, space="PSUM"))
        psD = P(tc.tile_pool(name="psD", bufs=2, space="PSUM"))
        psM = P(tc.tile_pool(name="psM", bufs=1, space="PSUM"))

        # ---------- constants ----------
        ones1 = const.tile([1, 128], dt.float32, tag="ones1")
        nc.gpsimd.memset(ones1[:], 1.0)
        ones1b = const.tile([1, 128], dt.bfloat16, tag="ones1b")
        nc.gpsimd.memset(ones1b[:], 1.0)
        mag = const.tile([128, 1], dt.float32, tag="mag")
        nc.gpsimd.memset(mag[:], MAGIC)
        nmag = const.tile([128, 1], dt.float32, tag="nmag")
        nc.gpsimd.memset(nmag[:], -MAGIC)
        wsc = const.tile([128, 16], dt.float32, tag="wsc")
        grow = const.tile([1, 16], dt.float32, tag="grow")
        nc.sync.dma_start(grow[:], wscin_ext[:, :])
        wsc_ps = psM.tile([128, 512], dt.float32, tag="psM")
        nc.tensor.matmul(wsc_ps[:, 0:16], ones1[:], grow[:], start=True, stop=True)
        nc.scalar.activation(wsc[:], wsc_ps[:, 0:16], AF.Copy)

        # ---------- rms_scale broadcast (bf16) ----------
        sbc = {}

        def bcast_scale(l):
            row = rsrow.tile([1, D], dt.bfloat16, tag="rsrow")
            nc.gpsimd.dma_start(row[:], rs_ext[l:l + 1, :])
            t = sbcpool.tile([128, D], dt.bfloat16, tag="sbc")
            for h in range(D // 512):
                ps = psM.tile([128, 512], dt.float32, tag="psM")
                nc.tensor.matmul(ps[:], ones1b[:], row[:, h * 512:(h + 1) * 512],
                                 start=True, stop=True)
                nc.scalar.activation(t[:, h * 512:(h + 1) * 512], ps[:], AF.Copy)
            sbc[l] = t

        bcast_scale(0)
        if n_layers > 1:
            bcast_scale(1)

        # ---------- phase A for layer 0 (transient x reads, row stats) ----------
        ssq = {0: batch.tile([128, ntt], dt.float32, tag="ssq", name="ssq0")}
        mxs = {0: batch.tile([128, ntt], dt.float32, tag="mx", name="mx0")}
        for i in range(ntt):
            xa = t1pool.tile([128, D], dt.float32, tag="t1")
            nc.sync.dma_start(xa[:], x_ext[i * 128:(i + 1) * 128, :])
            tb = t1pool.tile([128, D], dt.float32, tag="t1")
            nc.scalar.activation(tb[:], xa[:], AF.Square,
                                 accum_out=ssq[0][:, i:i + 1])
            nc.vector.tensor_tensor(tb[:], xa[:], sbc[0][:], op=ALU.mult)
            nc.vector.tensor_reduce(mxs[0][:, i:i + 1], tb[:],
                                    axis=mybir.AxisListType.X, op=ALU.max,
                                    apply_absolute_value=True)

        # ---------- batched row stats -> c1 (=s*rstd), rs (=1/s) ----------
        c1_all, rs_all, stats_t = {}, {}, {}

        def stats(l, lo, hi):
            if lo == 0:
                ms = batch.tile([128, ntt], dt.float32, tag="ms")
                rt = batch.tile([128, ntt], dt.float32, tag="rt")
                rstd = batch.tile([128, ntt], dt.float32, tag="rstd")
                nwt = batch.tile([128, ntt], dt.float32, tag="nwt")
                maxh = batch.tile([128, ntt], dt.float32, tag="maxh")
                sr = batch.tile([128, ntt], dt.float32, tag="sr")
                s_all = batch.tile([128, ntt], dt.float32, tag="s_all")
                c1 = batch.tile([128, ntt], dt.float32, tag="c1")
                rs = batch.tile([128, ntt], dt.float32, tag="rs_all")
                stats_t[l] = (ms, rt, rstd, nwt, maxh, sr, s_all, c1, rs)
                c1_all[l], rs_all[l] = c1, rs
            ms, rt, rstd, nwt, maxh, sr, s_all, c1, rs = stats_t[l]
            s_ = slice(lo, hi)
            nc.vector.tensor_scalar(ms[:, s_], ssq[l][:, s_], 1.0 / D, 1e-6, op0=ALU.mult, op1=ALU.add)
            nc.scalar.activation(rt[:, s_], ms[:, s_], AF.Sqrt)
            nc.vector.reciprocal(rstd[:, s_], rt[:, s_])
            # one Newton step fixes the Sqrt-LUT error that quantization
            # tie-flips amplify layer by layer
            nc.vector.tensor_tensor(nwt[:, s_], rstd[:, s_], rstd[:, s_], op=ALU.mult)
            nc.vector.tensor_tensor(nwt[:, s_], nwt[:, s_], ms[:, s_], op=ALU.mult)
            nc.vector.tensor_scalar(nwt[:, s_], nwt[:, s_], -0.5, 1.5, op0=ALU.mult, op1=ALU.add)
            nc.vector.tensor_tensor(rstd[:, s_], rstd[:, s_], nwt[:, s_], op=ALU.mult)
            nc.vector.tensor_tensor(maxh[:, s_], mxs[l][:, s_], rstd[:, s_], op=ALU.mult)
            nc.vector.tensor_scalar(maxh[:, s_], maxh[:, s_], 1e-5, None, op0=ALU.add)
            nc.vector.reciprocal(sr[:, s_], maxh[:, s_])
            nc.vector.tensor_scalar(s_all[:, s_], sr[:, s_], 127.0, 1e3, op0=ALU.mult, op1=ALU.min)
            nc.vector.tensor_scalar(s_all[:, s_], s_all[:, s_], 1e-3, None, op0=ALU.max)
            nc.vector.tensor_tensor(c1[:, s_], s_all[:, s_], rstd[:, s_], op=ALU.mult)
            nc.vector.reciprocal(rs[:, s_], s_all[:, s_])

        stats(0, 0, ntt)

        # ---------- q-pass ----------
        xs, qts = {}, {}

        def q_pass(l, i, xsrc):
            x1 = xpool.tile([128, D], dt.float32, tag="x1")
            nc.sync.dma_start(x1[:], xsrc[i * 128:(i + 1) * 128, :])
            xs[(l, i)] = x1
            t1 = t1pool.tile([128, D], dt.float32, tag="t1")
            nc.vector.tensor_tensor(t1[:], x1[:], sbc[l][:], op=ALU.mult)
            nc.scalar.activation(t1[:], t1[:], AF.Identity,
                                 scale=c1_all[l][:, i:i + 1], bias=mag[:])
            q = qpool.tile([128, D], dt.bfloat16, tag="q")
            nc.scalar.activation(q[:], t1[:], AF.Identity, bias=nmag[:])
            qT = qtpool.tile([128, NDK, 128], dt.bfloat16, tag="qt")
            nc.sync.dma_start_transpose(qT[:], q[:])
            qts[(l, i)] = qT

        q_pass(0, 0, x_ext)
        q_pass(0, 1, x_ext)

        # ---------- ternarize ([128, 1024] units) ----------
        pending_wr = []

        def flush_wr(keep=0):
            while len(pending_wr) > keep:
                dst, s8 = pending_wr.pop(0)
                nc.gpsimd.dma_start(dst, s8[:])

        def tern_unit(src_ap, idx, dst_sb=None, dst_dram=None):
            wt = wst.tile([128, 1024], dt.float32, tag="wst")
            nc.sync.dma_start(wt[:], src_ap)
            r = wi16.tile([128, 1024], dt.int16, tag="wi16")
            nc.vector.tensor_scalar(r[:], wt[:], wsc[:, idx:idx + 1], None, op0=ALU.mult)
            if dst_sb is not None:
                nc.vector.tensor_scalar(dst_sb, r[:], 1, -1, op0=ALU.min, op1=ALU.max)
            else:
                s8 = w8s.tile([128, 1024], dt.float8e4, tag="w8s")
                nc.vector.tensor_scalar(s8[:], r[:], 1, -1, op0=ALU.min, op1=ALU.max)
                pending_wr.append((dst_dram, s8))

        def tern_jobs(l, g8dst, u8dst, d8dst):
            # order: ch0 wg/wu, ch1, wd, ch2, ch3 so the first up-proj
            # f-chunks and the first down-proj unblock earliest at layer 0
            def wgu(ch):
                out = []
                for dk in range(NDK):
                    out.append((wg_ext[l, ch, dk], 3 * l,
                                g8dst[:, dk, ch * 1024:(ch + 1) * 1024]))
                    out.append((wu_ext[l, ch, dk], 3 * l + 1,
                                u8dst[:, dk, ch * 1024:(ch + 1) * 1024]))
                return out
            jobs = wgu(0) + wgu(1)
            for ft in range(NFT):
                jobs.append((wd_ext[l, ft * 128:(ft + 1) * 128, :], 3 * l + 2,
                             d8dst[:, ft, :]))
            jobs += wgu(2) + wgu(3)
            return jobs

        # layer-0: ternarize straight into the resident fp8 tiles
        wg_t = wpool.tile([128, NDK, F], dt.float8e4, tag="wts")
        wu_t = wpool.tile([128, NDK, F], dt.float8e4, tag="wts")
        wd_t = wpool.tile([128, NFT, D], dt.float8e4, tag="wts")
        for src_ap, idx, dst in tern_jobs(0, wg_t, wu_t, wd_t):
            tern_unit(src_ap, idx, dst_sb=dst)

        # ---------- main layer loop ----------
        xbuf = {}
        stage8 = {}
        for l in range(n_layers):
            if l > 0:
                g8, u8, d8 = stage8[l]
                wg_t = wpool.tile([128, NDK, F], dt.float8e4, tag="wts")
                wu_t = wpool.tile([128, NDK, F], dt.float8e4, tag="wts")
                wd_t = wpool.tile([128, NFT, D], dt.float8e4, tag="wts")
                nc.gpsimd.dma_start(wg_t[:], g8[:])
                nc.gpsimd.dma_start(wu_t[:], u8[:])
                nc.gpsimd.dma_start(wd_t[:], d8[:])
                xsrc = xbuf[l - 1]
            else:
                xsrc = x_ext

            if l + 1 < n_layers:
                ssq[l + 1] = batch.tile([128, ntt], dt.float32, tag="ssq", name="ssqn")
                mxs[l + 1] = batch.tile([128, ntt], dt.float32, tag="mx", name="mxn")
                if l + 1 > 1:
                    bcast_scale(l + 1)
                g8 = dram.tile([128, NDK, F], dt.float8e4, tag="wg8")
                u8 = dram.tile([128, NDK, F], dt.float8e4, tag="wu8")
                d8 = dram.tile([128, NFT, D], dt.float8e4, tag="wd8")
                stage8[l + 1] = (g8, u8, d8)
                jobs = tern_jobs(l + 1, g8, u8, d8)
            else:
                jobs = []

            if l == n_layers - 1:
                xdst = out_ext
            else:
                xdst = dram.tile([tok, D], dt.float32, tag="xbuf")
                xbuf[l] = xdst

            # tern jobs spread over tile slots 2..ntt-1, interleaved per fg
            nslots = ntt - 2
            jper = (len(jobs) + nslots - 1) // nslots if jobs else 0

            up_state = {}   # (l, i) -> [x1, stile, g, gq]  until magic chunks done
            pend = {}       # (l, i) -> (x1, stile, gqT)    until down-proj drains

            def magic_chunk(l_, i_, cs):
                # round chunk [cs*512, (cs+1)*512) of tile i_'s g via the
                # 2-op magic-add trick; runs between the next tile's silus
                x1, stile, g, gq = up_state[(l_, i_)]
                a, b = cs * 512, (cs + 1) * 512
                nc.scalar.activation(g[:, a:b], g[:, a:b], AF.Identity,
                                     scale=stile[:, 14:15], bias=mag[:])
                nc.scalar.activation(gq[:, a:b], g[:, a:b], AF.Identity, bias=nmag[:])

            def finish_gq(l_, i_):
                x1, stile, g, gq = up_state.pop((l_, i_))
                gqT = gqtpool.tile([128, NFT, 128], dt.bfloat16, tag="gqt")
                nc.sync.dma_start_transpose(gqT[:], gq[:])
                pend[(l_, i_)] = (x1, stile, gqT)

            def emit_down(l_, i_):
                x1, stile, gqT = pend.pop((l_, i_))
                xd0 = psD.tile([128, 512], dt.float32, tag="xdps")
                xd1 = psD.tile([128, 512], dt.float32, tag="xdps")
                for ft in range(NFT):
                    nc.tensor.matmul(xd0[:], gqT[:, ft, :], wd_t[:, ft, 0:512],
                                     start=(ft == 0), stop=(ft == NFT - 1))
                    nc.tensor.matmul(xd1[:], gqT[:, ft, :], wd_t[:, ft, 512:1024],
                                     start=(ft == 0), stop=(ft == NFT - 1))
                for dc, xd_ps in ((0, xd0), (1, xd1)):
                    xdr = xdrpool.tile([128, 512], dt.float32, tag="xdr")
                    nc.scalar.activation(xdr[:], xd_ps[:], AF.Copy,
                                         scale=stile[:, 15:16])
                    nc.vector.tensor_tensor(
                        x1[:, dc * 512:(dc + 1) * 512],
                        x1[:, dc * 512:(dc + 1) * 512], xdr[:], op=ALU.add)
                nc.sync.dma_start(xdst[i_ * 128:(i_ + 1) * 128, :], x1[:])
                if l_ + 1 < n_layers:
                    t1 = t1pool.tile([128, D], dt.float32, tag="t1")
                    nc.scalar.activation(t1[:], x1[:], AF.Square,
                                         accum_out=ssq[l_ + 1][:, i_:i_ + 1])
                    nc.vector.tensor_tensor(t1[:], x1[:], sbc[l_ + 1][:], op=ALU.mult)
                    nc.vector.tensor_reduce(mxs[l_ + 1][:, i_:i_ + 1], t1[:],
                                            axis=mybir.AxisListType.X, op=ALU.max,
                                            apply_absolute_value=True)

            for i in range(ntt):
                x1 = xs.pop((l, i))
                qT = qts.pop((l, i))

                # ---- up-projection + GLU, interleaved with tile i-1's magic
                #      chunks and the tern units for layer l+1 ----
                g = gpool.tile([128, F], dt.float32, tag="g")
                prev = (l, i - 1) in up_state
                if prev:
                    gq = gqpool.tile([128, F], dt.bfloat16, tag="gq")
                    up_state[(l, i - 1)][3] = gq
                stile = sc.tile([128, 16], dt.float32, tag="stile")
                slot_jobs = jobs[(i - 2) * jper:(i - 1) * jper] if i >= 2 else []
                nj = (len(slot_jobs) + NFC - 1) // NFC if slot_jobs else 0
                for fg in range(NFC):
                    u_ps = psA.tile([128, 512], dt.float32, tag="ups")
                    v_ps = psB.tile([128, 512], dt.float32, tag="vps")
                    for dk in range(NDK):
                        nc.tensor.matmul(
                            u_ps[:], qT[:, dk, :], wg_t[:, dk, fg * 512:(fg + 1) * 512],
                            start=(dk == 0), stop=(dk == NDK - 1))
                        nc.tensor.matmul(
                            v_ps[:], qT[:, dk, :], wu_t[:, dk, fg * 512:(fg + 1) * 512],
                            start=(dk == 0), stop=(dk == NDK - 1))
                    su = silupool.tile([128, 512], dt.float32, tag="silu")
                    nc.scalar.activation(su[:], u_ps[:], AF.Silu,
                                         scale=rs_all[l][:, i:i + 1])
                    if prev:
                        magic_chunk(l, i - 1, fg)
                    nc.vector.tensor_tensor(
                        g[:, fg * 512:(fg + 1) * 512], su[:], v_ps[:], op=ALU.mult)
                    nc.vector.tensor_reduce(
                        stile[:, fg:fg + 1], g[:, fg * 512:(fg + 1) * 512],
                        axis=mybir.AxisListType.X, op=ALU.max,
                        apply_absolute_value=True)

                # ---- s2 = clip(127/(max|g|/s + 1e-5)); c2 = s2/s; rs2 = 1/s2 ----
                nc.vector.tensor_reduce(
                    stile[:, 8:9], stile[:, 0:8], axis=mybir.AxisListType.X,
                    op=ALU.max, apply_absolute_value=False)
                nc.vector.tensor_tensor(stile[:, 9:10], stile[:, 8:9],
                                        rs_all[l][:, i:i + 1], op=ALU.mult)
                nc.vector.tensor_scalar(stile[:, 10:11], stile[:, 9:10], 1e-5, None,
                                        op0=ALU.add)
                nc.vector.reciprocal(stile[:, 11:12], stile[:, 10:11])
                nc.vector.tensor_scalar(stile[:, 12:13], stile[:, 11:12], 127.0, 1e3,
                                        op0=ALU.mult, op1=ALU.min)
                nc.vector.tensor_scalar(stile[:, 13:14], stile[:, 12:13], 1e-3, None,
                                        op0=ALU.max)
                nc.vector.tensor_tensor(stile[:, 14:15], stile[:, 13:14],
                                        rs_all[l][:, i:i + 1], op=ALU.mult)
                nc.vector.reciprocal(stile[:, 15:16], stile[:, 13:14])
                up_state[(l, i)] = [x1, stile, g, None]

                # ---- tile i-2's down-projection; then tile i-1's transpose ----
                if i >= 2:
                    emit_down(l, i - 2)
                if i >= 1:
                    finish_gq(l, i - 1)

                if i + PRE < ntt:
                    q_pass(l, i + PRE, xsrc)
                for src_ap, idx, dst in slot_jobs:
                    flush_wr(keep=1)
                    tern_unit(src_ap, idx, dst_dram=dst)
                if i == ntt - 1:
                    flush_wr(keep=0)
                if l + 1 < n_layers:
                    # tails lag 2 slots behind up-proj: half-1 stats are safe
                    # once slot ntt//2+1's emit_down has run
                    half1_slot = ntt // 2 + 1
                    if i == half1_slot:
                        stats(l + 1, 0, ntt // 2)
                    if i == max(ntt - 2, half1_slot):
                        q_pass(l + 1, 0, xdst)
                    if i == ntt - 1:
                        q_pass(l + 1, 1, xdst)

            # drain the tail: tile ntt-1's magic chunks, transpose, downs
            gq = gqpool.tile([128, F], dt.bfloat16, tag="gq")
            up_state[(l, ntt - 1)][3] = gq
            for cs in range(NFC):
                magic_chunk(l, ntt - 1, cs)
            emit_down(l, ntt - 2)
            finish_gq(l, ntt - 1)
            emit_down(l, ntt - 1)
            if l + 1 < n_layers:
                stats(l + 1, ntt // 2, ntt)

    _split_excess_waits(nc)
    return nc


_nc_cache = {}


def _get_nc(key=(NCORES, NTT, L)):
    if key not in _nc_cache:
        _nc_cache[key] = build(*key)
    return _nc_cache[key]


def _repack(w, n_layers):
    # [L, D, F] -> [L, F//1024, D//128, 128, 1024]: each (ch, dk) tile is one
    # contiguous 512 KB stream unit
    return np.ascontiguousarray(
        w.reshape(n_layers, D // 128, 128, F // 1024, 1024).transpose(0, 3, 1, 2, 4)
    )


def _make_in_maps(x, rs, wg, wu, wd, n_cores=NCORES):
    n_layers = rs.shape[0]
    wg_r = _repack(wg, n_layers)
    wu_r = _repack(wu, n_layers)
    wscin = np.zeros((1, 16), dtype=np.float32)
    for l in range(n_layers):
        for mi, w in enumerate((wg, wu, wd)):
            m = np.float32(np.abs(w[l]).max())
            wscin[0, 3 * l + mi] = np.float32(127.0) / (m + np.float32(1e-5))
    in_maps = []
    for c in range(n_cores):
        in_maps.append({
            "x": x[c],
            "rs": rs,
            "wg": wg_r,
            "wu": wu_r,
            "wd": wd,
            "wscin": wscin,
        })
    return in_maps


def kernel(x, rms_scale, W_g, W_u, W_d):
    """Full-input entry point: shard over batch, run 8-core SPMD, gather."""
    x = np.ascontiguousarray(np.asarray(x, dtype=np.float32))
    rs = np.ascontiguousarray(np.asarray(rms_scale, dtype=np.float32))
    wg = np.ascontiguousarray(np.asarray(W_g, dtype=np.float32))
    wu = np.ascontiguousarray(np.asarray(W_u, dtype=np.float32))
    wd = np.ascontiguousarray(np.asarray(W_d, dtype=np.float32))
    B, S, Dx = x.shape
    assert (B, S, Dx) == (NCORES, TOK, D), (B, S, Dx)
    nc = _get_nc()
    in_maps = _make_in_maps(x, rs, wg, wu, wd)
    res = run_bass_kernel_spmd(nc, in_maps, list(range(NCORES)))
    return np.stack([res.results[c]["out"] for c in range(NCORES)], axis=0)
